# revision 1
# baseline (speedup 1.0000x reference)
"""Trainium2 Bass kernel for a local-attention transformer block.

Strategy: data-parallel over tokens. B*S = 2*4096 = 8192 tokens are split
into 8 shards of 1024 tokens (4 shards per batch element, so no shard
crosses a batch boundary). Each core gets its shard plus a 128-token halo
of preceding tokens (zeros at batch start), computes QKV for the halo'd
range, runs banded sliding-window attention (each 128-query block attends
to exactly two 128-key blocks), out-projection, LN1, FFN (exact gelu),
LN2 — entirely locally, no collectives. Matmuls run in bf16 with fp32
accumulation; softmax/layernorm/residual paths stay fp32.
"""

import numpy as np
import ml_dtypes
from contextlib import nullcontext as _nullctx

# ---- problem constants (hardcoded per contract) ----
B, S, D = 2, 4096, 768
NH, HD = 12, 64
DFF = 4 * D            # 3072
DQK = 2 * D            # 1536 (Q and K channels)
WIN = 128              # sliding window
EPS = 1e-5
T = 128                # tile (partition) size
NB = 8                 # own 128-token blocks per core
NBH = NB + 1           # with one halo block
NTOK = NB * T          # 1024 own tokens per core
NTOKH = NBH * T        # 1152 with halo
ND = D // T            # 6
NC2 = DQK // T         # 12
NF = DFF // T          # 24
N_CORES = 8
NEG = -1e30

_CACHE = {}


def _build_nc(act="gelu", reps=1, loop=1):
    import concourse.bacc as bacc
    import concourse.mybir as mybir
    from concourse import tile
    from concourse.masks import make_identity
    from contextlib import ExitStack

    f32 = mybir.dt.float32
    bf16 = mybir.dt.bfloat16
    AF = mybir.ActivationFunctionType
    ALU = mybir.AluOpType
    AX = mybir.AxisListType

    nc = bacc.Bacc("TRN2", target_bir_lowering=False, debug=False,
                   num_devices=N_CORES)

    # ---- DRAM I/O ----
    xh_d = nc.dram_tensor("xh", [NTOKH, D], f32, kind="ExternalInput").ap()
    mf_d = nc.dram_tensor("mask_first", [T, 2 * T], f32, kind="ExternalInput").ap()
    mr_d = nc.dram_tensor("mask_rest", [T, 2 * T], f32, kind="ExternalInput").ap()
    wqk_d = nc.dram_tensor("wqkT", [D, DQK], bf16, kind="ExternalInput").ap()
    wv_d = nc.dram_tensor("wvT", [D, D], bf16, kind="ExternalInput").ap()
    wo_d = nc.dram_tensor("woT", [D, D], bf16, kind="ExternalInput").ap()
    w1_d = nc.dram_tensor("w1T", [D, DFF], bf16, kind="ExternalInput").ap()
    w2_d = nc.dram_tensor("w2T", [DFF, D], bf16, kind="ExternalInput").ap()
    qkb_d = nc.dram_tensor("qkb", [T, NC2], f32, kind="ExternalInput").ap()
    b1c_d = nc.dram_tensor("b1c", [T, NF], f32, kind="ExternalInput").ap()
    # row vectors replicated to 128 partitions on host
    rep_names = ["vb", "ob", "b2", "g1", "bb1", "g2", "bb2"]
    reps_d = {n: nc.dram_tensor(f"rep_{n}", [T, D], f32, kind="ExternalInput").ap()
              for n in rep_names}
    out_d = nc.dram_tensor("out", [NTOK, D], f32, kind="ExternalOutput").ap()

    with tile.TileContext(nc) as tc:
      with (tc.For_i(0, loop, 1) if loop > 1 else _nullctx()):
        for rep in range(reps):
          ctx = ExitStack()
          persist = ctx.enter_context(tc.tile_pool(name=f"persist{rep}", bufs=1))
          ident = persist.tile([T, T], bf16, tag="ident")
          make_identity(nc, ident[:])
          yT_all = persist.tile([T, NB, ND, T], bf16, tag="yT")
          mf_sb = persist.tile([T, 2 * T], f32, tag="mf")
          nc.sync.dma_start(mf_sb[:], mf_d[:])
          mr_sb = persist.tile([T, 2 * T], f32, tag="mr")
          nc.sync.dma_start(mr_sb[:], mr_d[:])
          qkb_sb = persist.tile([T, NC2], f32, tag="qkb")
          nc.sync.dma_start(qkb_sb[:], qkb_d[:])
          b1c_sb = persist.tile([T, NF], f32, tag="b1c")
          nc.sync.dma_start(b1c_sb[:], b1c_d[:])
          eps_sb = persist.tile([T, 1], f32, tag="eps")
          nc.gpsimd.memset(eps_sb[:], EPS)
          rep_sb = {}
          for n in rep_names:
              rep_sb[n] = persist.tile([T, D], f32, tag=f"rep_{n}",
                                       name=f"rep_{n}_sb")
              nc.sync.dma_start(rep_sb[n][:], reps_d[n][:])

          def emit_ln(pool, ps_pool, xin, g_rep, b_rep, out_tag):
              ns = pool.tile([T, 1], f32, tag="ln_ns")
              nc.vector.tensor_reduce(ns[:], xin[:], axis=AX.X, op=ALU.add,
                                      negate=True)
              nm = pool.tile([T, 1], f32, tag="ln_nm")
              nc.scalar.mul(nm[:], ns[:], 1.0 / D)
              xc = pool.tile([T, D], f32, tag="ln_xc")
              nc.vector.tensor_scalar_add(xc[:], xin[:], nm[:])
              sq = pool.tile([T, D], f32, tag="ln_sq")
              vs = pool.tile([T, 1], f32, tag="ln_vs")
              nc.vector.scalar_tensor_tensor(sq[:], xc[:], 1.0, xc[:],
                                             op0=ALU.mult, op1=ALU.mult,
                                             accum_out=vs[:])
              std = pool.tile([T, 1], f32, tag="ln_std")
              nc.scalar.activation(std[:], vs[:], AF.Sqrt, bias=eps_sb[:],
                                   scale=1.0 / D)
              rstd = pool.tile([T, 1], f32, tag="ln_rstd")
              nc.vector.reciprocal(rstd[:], std[:])
              xg = pool.tile([T, D], f32, tag="ln_xg")
              nc.vector.scalar_tensor_tensor(xg[:], xc[:], rstd[:], g_rep[:],
                                             op0=ALU.mult, op1=ALU.mult)
              out = pool.tile([T, D], f32, tag=out_tag)
              nc.vector.tensor_tensor(out[:], xg[:], b_rep[:], op=ALU.add)
              return out

          # ================= phase A: QKV generation + attention =============
          with tc.tile_pool(name=f"wA{rep}", bufs=1) as wA, \
               tc.tile_pool(name=f"kv{rep}", bufs=1) as kv:
              wqk_sb = wA.tile([T, ND, DQK], bf16, tag="wqk")
              nc.sync.dma_start(wqk_sb[:], wqk_d.rearrange("(j p) n -> p j n", p=T))
              wv_sb = wA.tile([T, ND, D], bf16, tag="wv")
              nc.sync.dma_start(wv_sb[:], wv_d.rearrange("(j p) n -> p j n", p=T))
              qkT_sb = kv.tile([T, NC2, NTOKH], bf16, tag="qkT")
              v_sb = kv.tile([T, NBH, D], bf16, tag="v")

              with tc.tile_pool(name=f"workA{rep}", bufs=2) as workA, \
                   tc.tile_pool(name=f"psA{rep}", bufs=2, space="PSUM") as psA:
                  for i in range(NBH):
                      x_sb = workA.tile([T, D], f32, tag="x")
                      nc.sync.dma_start(x_sb[:], xh_d[i * T:(i + 1) * T, :])
                      xb = workA.tile([T, D], bf16, tag="xb")
                      nc.vector.tensor_copy(xb[:], x_sb[:])
                      xT = workA.tile([T, ND, T], bf16, tag="xT")
                      for j in range(ND):
                          ptr = psA.tile([T, T], bf16, tag="tr")
                          nc.tensor.transpose(ptr[:], xb[:, j * T:(j + 1) * T],
                                              ident[:])
                          nc.vector.tensor_copy(xT[:, j, :], ptr[:])
                      # Q,K in [channel, token] layout
                      for ci in range(NC2):
                          pqk = psA.tile([T, T], f32, tag="qk")
                          for j in range(ND):
                              nc.tensor.matmul(pqk[:],
                                               wqk_sb[:, j, ci * T:(ci + 1) * T],
                                               xT[:, j, :],
                                               start=(j == 0), stop=(j == ND - 1))
                          nc.scalar.activation(qkT_sb[:, ci, i * T:(i + 1) * T],
                                               pqk[:], AF.Identity,
                                               bias=qkb_sb[:, ci:ci + 1])
                      # V in [token, channel] layout
                      for nh in range(2):
                          sl = slice(nh * 384, (nh + 1) * 384)
                          pv = psA.tile([T, 384], f32, tag="v")
                          for j in range(ND):
                              nc.tensor.matmul(pv[:], xT[:, j, :],
                                               wv_sb[:, j, sl],
                                               start=(j == 0), stop=(j == ND - 1))
                          nc.vector.tensor_tensor(v_sb[:, i, sl], pv[:],
                                                  rep_sb["vb"][:, sl], op=ALU.add)

              # ---- banded attention: query block t sees key blocks t, t+1 ----
              with tc.tile_pool(name=f"attnA{rep}", bufs=3) as attnA, \
                   tc.tile_pool(name=f"psS{rep}", bufs=2, space="PSUM") as psS:
                  for t in range(NB):
                      msk = mf_sb if t == 0 else mr_sb
                      for h in range(NH):
                          ci = h // 2
                          po = (h % 2) * HD
                          ps_s = psS.tile([T, 2 * T], f32, tag="s")
                          qT = qkT_sb[po:po + HD, ci, (t + 1) * T:(t + 2) * T]
                          kT = qkT_sb[po:po + HD, ND + ci, t * T:(t + 2) * T]
                          nc.tensor.matmul(ps_s[:], qT, kT, start=True, stop=True)
                          S_sb = attnA.tile([T, 2 * T], f32, tag="S")
                          nc.vector.tensor_tensor(S_sb[:], ps_s[:], msk[:],
                                                  op=ALU.add)
                          P = attnA.tile([T, 2 * T], bf16, tag="P")
                          den = attnA.tile([T, 1], f32, tag="den")
                          nc.scalar.activation(P[:], S_sb[:], AF.Exp,
                                               scale=0.125, accum_out=den[:])
                          rec = attnA.tile([T, 1], f32, tag="rec")
                          nc.vector.reciprocal(rec[:], den[:])
                          Pn = attnA.tile([T, 2 * T], bf16, tag="Pn")
                          nc.vector.tensor_scalar_mul(Pn[:], P[:], rec[:])
                          ps_pt = psS.tile([T, 2 * T], bf16, tag="pt")
                          nc.tensor.transpose(ps_pt[:, 0:T], Pn[:, 0:T], ident[:])
                          nc.tensor.transpose(ps_pt[:, T:2 * T], Pn[:, T:2 * T],
                                              ident[:])
                          PT = attnA.tile([T, 2 * T], bf16, tag="PT")
                          nc.scalar.copy(PT[:], ps_pt[:])
                          ps_y = psS.tile([HD, T], f32, tag="y")
                          nc.tensor.matmul(ps_y[:],
                                           v_sb[:, t, h * HD:(h + 1) * HD],
                                           PT[:, 0:T], start=True, stop=False)
                          nc.tensor.matmul(ps_y[:],
                                           v_sb[:, t + 1, h * HD:(h + 1) * HD],
                                           PT[:, T:2 * T], start=False, stop=True)
                          nc.vector.tensor_copy(
                              yT_all[po:po + HD, t, ci, :], ps_y[:])

          # ============ phase B: out-proj + LN1 + FFN + LN2 ==================
          with tc.tile_pool(name=f"wB{rep}", bufs=1) as wB:
              wo_sb = wB.tile([T, ND, D], bf16, tag="wo")
              nc.sync.dma_start(wo_sb[:], wo_d.rearrange("(j p) n -> p j n", p=T))
              w1_sb = wB.tile([T, ND, DFF], bf16, tag="w1")
              nc.sync.dma_start(w1_sb[:], w1_d.rearrange("(j p) n -> p j n", p=T))
              w2_sb = wB.tile([T, NF, D], bf16, tag="w2")
              nc.sync.dma_start(w2_sb[:], w2_d.rearrange("(j p) n -> p j n", p=T))

              with tc.tile_pool(name=f"workB{rep}", bufs=2) as workB, \
                   tc.tile_pool(name=f"psB{rep}", bufs=2, space="PSUM") as psB:
                  for t in range(NB):
                      xo = workB.tile([T, D], f32, tag="xo")
                      nc.sync.dma_start(xo[:], xh_d[(t + 1) * T:(t + 2) * T, :])
                      x1pre = workB.tile([T, D], f32, tag="x1pre")
                      for nh in range(2):
                          sl = slice(nh * 384, (nh + 1) * 384)
                          pz = psB.tile([T, 384], f32, tag="mm")
                          for j in range(ND):
                              nc.tensor.matmul(pz[:], yT_all[:, t, j, :],
                                               wo_sb[:, j, sl],
                                               start=(j == 0), stop=(j == ND - 1))
                          nc.vector.tensor_tensor(x1pre[:, sl], pz[:], xo[:, sl],
                                                  op=ALU.add)
                          nc.vector.tensor_tensor(x1pre[:, sl], x1pre[:, sl],
                                                  rep_sb["ob"][:, sl], op=ALU.add)
                      x1 = emit_ln(workB, psB, x1pre, rep_sb["g1"], rep_sb["bb1"],
                                   "x1")
                      x1b = workB.tile([T, D], bf16, tag="x1b")
                      nc.vector.tensor_copy(x1b[:], x1[:])
                      x1T = workB.tile([T, ND, T], bf16, tag="x1T")
                      for j in range(ND):
                          ptr = psB.tile([T, T], bf16, tag="tr")
                          nc.tensor.transpose(ptr[:], x1b[:, j * T:(j + 1) * T],
                                              ident[:])
                          nc.vector.tensor_copy(x1T[:, j, :], ptr[:])
                      h_sb = workB.tile([T, NF, T], bf16, tag="h")
                      for fi in range(NF):
                          ph = psB.tile([T, T], f32, tag="h1")
                          for j in range(ND):
                              nc.tensor.matmul(ph[:],
                                               w1_sb[:, j, fi * T:(fi + 1) * T],
                                               x1T[:, j, :],
                                               start=(j == 0), stop=(j == ND - 1))
                          act_fn = AF.Gelu if act == "gelu" else AF.Identity
                          nc.scalar.activation(h_sb[:, fi, :], ph[:], act_fn,
                                               bias=b1c_sb[:, fi:fi + 1])
                      x2pre = workB.tile([T, D], f32, tag="x2pre")
                      for nh in range(2):
                          sl = slice(nh * 384, (nh + 1) * 384)
                          pz2 = psB.tile([T, 384], f32, tag="mm")
                          for fi in range(NF):
                              nc.tensor.matmul(pz2[:], h_sb[:, fi, :],
                                               w2_sb[:, fi, sl],
                                               start=(fi == 0),
                                               stop=(fi == NF - 1))
                          nc.vector.tensor_tensor(x2pre[:, sl], pz2[:], x1[:, sl],
                                                  op=ALU.add)
                          nc.vector.tensor_tensor(x2pre[:, sl], x2pre[:, sl],
                                                  rep_sb["b2"][:, sl], op=ALU.add)
                      out_sb = emit_ln(workB, psB, x2pre, rep_sb["g2"],
                                       rep_sb["bb2"], "outb")
                      nc.sync.dma_start(out_d[t * T:(t + 1) * T, :], out_sb[:])

          ctx.close()

    nc.compile()
    return nc


def _get_nc(act="gelu", reps=1, loop=1):
    key = (act, reps, loop)
    if key not in _CACHE:
        _CACHE[key] = _build_nc(act, reps, loop)
    return _CACHE[key]


def make_in_maps(x, in_proj_w, in_proj_b, out_w, out_b, ff_w1, ff_b1,
                 ff_w2, ff_b2, n1_g, n1_b, n2_g, n2_b):
    bf = ml_dtypes.bfloat16
    f32 = np.float32
    x = np.asarray(x, f32).reshape(B, S, D)

    shared = {
        "wqkT": np.ascontiguousarray(np.asarray(in_proj_w, f32)[:DQK].T).astype(bf),
        "wvT": np.ascontiguousarray(np.asarray(in_proj_w, f32)[DQK:].T).astype(bf),
        "woT": np.ascontiguousarray(np.asarray(out_w, f32).T).astype(bf),
        "w1T": np.ascontiguousarray(np.asarray(ff_w1, f32).T).astype(bf),
        "w2T": np.ascontiguousarray(np.asarray(ff_w2, f32).T).astype(bf),
        "qkb": np.ascontiguousarray(
            np.asarray(in_proj_b, f32)[:DQK].reshape(NC2, T).T),
        "b1c": np.ascontiguousarray(np.asarray(ff_b1, f32).reshape(NF, T).T),
    }
    for name, vec in [("vb", np.asarray(in_proj_b, f32)[DQK:]),
                      ("ob", out_b), ("b2", ff_b2), ("g1", n1_g),
                      ("bb1", n1_b), ("g2", n2_g), ("bb2", n2_b)]:
        shared[f"rep_{name}"] = np.ascontiguousarray(
            np.broadcast_to(np.asarray(vec, f32)[None, :], (T, D)))

    q = np.arange(T, dtype=np.int64)[:, None]
    k = np.arange(T, dtype=np.int64)[None, :]
    M0 = np.where(k > q, 0.0, NEG).astype(f32)
    M1 = np.where(k <= q, 0.0, NEG).astype(f32)
    mask_rest = np.ascontiguousarray(np.concatenate([M0, M1], axis=1))
    mask_first_bs = np.ascontiguousarray(
        np.concatenate([np.full((T, T), NEG, f32), M1], axis=1))

    in_maps = []
    for c in range(N_CORES):
        b, i0 = divmod(c * NTOK, S)
        halo = (np.zeros((T, D), f32) if i0 == 0
                else x[b, i0 - T:i0])
        xh = np.ascontiguousarray(
            np.concatenate([halo, x[b, i0:i0 + NTOK]], axis=0))
        m = dict(shared)
        m["xh"] = xh
        m["mask_first"] = mask_first_bs if i0 == 0 else mask_rest
        m["mask_rest"] = mask_rest
        in_maps.append(m)
    return in_maps


def kernel(**inputs):
    from concourse.bass_utils import run_bass_kernel_spmd
    nc = _get_nc()
    in_maps = make_in_maps(**inputs)
    res = run_bass_kernel_spmd(nc, in_maps, core_ids=list(range(N_CORES)))
    outs = [res.results[c]["out"] for c in range(N_CORES)]
    return np.concatenate(outs, axis=0).reshape(B, S, D).astype(np.float32)



# revision 15
# speedup vs baseline: 1.5936x; 1.5936x over previous
"""Trainium2 Bass kernel for a local-attention transformer block.

Data-parallel over tokens: 8 shards of 1024 tokens (+128-token halo).
Per core: transpose x to [d,tok]; QKV with large moving dims; attention
in transposed-score orientation (exp emits P^T directly, denominator via
an appended ones-column in the PV matmul, mask applied multiplicatively
after exp); out-proj + LN1 interleaved with attention; FFN with N=512
moving dims and gelu bias folded with n1_b@W1^T. Biases enter PSUM as
rank-1 matmuls or per-partition activation biases. Matmuls bf16 with
fp32 accumulation; softmax/layernorm kept fp32.
"""

import numpy as np
import ml_dtypes

# ---- problem constants (hardcoded per contract) ----
B, S, D = 2, 4096, 768
NH, HD = 12, 64
DFF = 4 * D            # 3072
DQK = 2 * D            # 1536
WIN = 128
EPS = 1e-5
T = 128
NB = 8                 # own 128-token blocks per core
NBH = NB + 1           # with halo block
NTOK = NB * T          # 1024
NTOKH = NBH * T        # 1152
ND = D // T            # 6
NF = DFF // T          # 24
N_CORES = 8

_CACHE = {}


def _build_nc(act="gelu"):
    import concourse.bacc as bacc
    import concourse.mybir as mybir
    from concourse import tile
    from concourse.masks import make_identity

    f32 = mybir.dt.float32
    bf16 = mybir.dt.bfloat16
    AF = mybir.ActivationFunctionType
    ALU = mybir.AluOpType

    nc = bacc.Bacc("TRN2", target_bir_lowering=False, debug=False,
                   num_devices=N_CORES)

    # ---- DRAM I/O ----
    xh_d = nc.dram_tensor("xh", [NTOKH, D], bf16, kind="ExternalInput").ap()
    wqk_d = nc.dram_tensor("wqkT", [D, DQK], bf16, kind="ExternalInput").ap()
    wv_d = nc.dram_tensor("wvT", [D, D], bf16, kind="ExternalInput").ap()
    wo_d = nc.dram_tensor("woT", [D, D], bf16, kind="ExternalInput").ap()
    w1_d = nc.dram_tensor("w1T", [D, DFF], bf16, kind="ExternalInput").ap()
    w2_d = nc.dram_tensor("w2T", [DFF, D], bf16, kind="ExternalInput").ap()
    qkb_d = nc.dram_tensor("qkb", [T, 2 * ND], f32, kind="ExternalInput").ap()
    gelub_d = nc.dram_tensor("gelub", [T, NF], f32, kind="ExternalInput").ap()
    obrep_d = nc.dram_tensor("obrep", [T, D], bf16, kind="ExternalInput").ap()
    b2rep_d = nc.dram_tensor("b2rep", [T, D], bf16, kind="ExternalInput").ap()
    g1rep_d = nc.dram_tensor("g1rep", [T, D], f32, kind="ExternalInput").ap()
    g2rep_d = nc.dram_tensor("g2rep", [T, D], f32, kind="ExternalInput").ap()
    n2brep_d = nc.dram_tensor("n2brep", [T, D], f32, kind="ExternalInput").ap()
    # masks in transposed [k, (h4, kb, q)] orientation, 0/1, tiled for 4 heads
    m01f_d = nc.dram_tensor("m01f", [T, 4 * 2 * T], bf16, kind="ExternalInput").ap()
    m01r_d = nc.dram_tensor("m01r", [T, 4 * 2 * T], bf16, kind="ExternalInput").ap()
    out_d = nc.dram_tensor("out", [NTOK, D], f32, kind="ExternalOutput").ap()

    with tile.TileContext(nc) as tc:
      with tc.tile_pool(name="persist", bufs=1) as persist, \
           tc.tile_pool(name="mid", bufs=1) as mid:
        ident = persist.tile([T, T], bf16, tag="ident")
        make_identity(nc, ident[:])
        ones_col = persist.tile([T, 1], bf16, tag="ones_col")
        nc.gpsimd.memset(ones_col[:], 1.0)
        # E0: row 0 all-ones selector for rank-1 bias adds (K=128)
        e0mat = persist.tile([T, T], bf16, tag="e0mat")
        nc.gpsimd.memset(e0mat[:], 0.0)
        nc.gpsimd.memset(e0mat[0:1, :], 1.0)
        eps_sb = persist.tile([T, 1], f32, tag="eps")
        nc.gpsimd.memset(eps_sb[:], EPS)
        qkb_sb = persist.tile([T, 2 * ND], f32, tag="qkb")
        nc.sync.dma_start(qkb_sb[:], qkb_d[:])
        gelub_sb = persist.tile([T, NF], f32, tag="gelub")
        nc.sync.dma_start(gelub_sb[:], gelub_d[:])
        obrep_sb = persist.tile([T, D], bf16, tag="obrep")
        nc.sync.dma_start(obrep_sb[:], obrep_d[:])
        b2rep_sb = persist.tile([T, D], bf16, tag="b2rep")
        nc.sync.dma_start(b2rep_sb[:], b2rep_d[:])
        g1rep_sb = persist.tile([T, D], f32, tag="g1rep")
        nc.sync.dma_start(g1rep_sb[:], g1rep_d[:])
        g2rep_sb = persist.tile([T, D], f32, tag="g2rep")
        nc.sync.dma_start(g2rep_sb[:], g2rep_d[:])
        n2brep_sb = persist.tile([T, D], f32, tag="n2brep")
        nc.sync.dma_start(n2brep_sb[:], n2brep_d[:])
        m01f_sb = persist.tile([T, 8 * T], bf16, tag="m01f")
        nc.sync.dma_start(m01f_sb[:], m01f_d[:])
        m01r_sb = persist.tile([T, 8 * T], bf16, tag="m01r")
        nc.sync.dma_start(m01r_sb[:], m01r_d[:])
        xh_sb = persist.tile([T, NBH, D], bf16, tag="xh")
        nc.sync.dma_start(xh_sb[:], xh_d.rearrange("(b p) d -> p b d", p=T))
        # w1 preloaded early (tile only; DMA emitted after phase-A weights
        # so it doesn't delay them in the DMA queue)
        w1_sb = persist.tile([T, ND, DFF], bf16, tag="w1")

        x1_all = mid.tile([T, NB, D], bf16, tag="x1")
        x1T_all = mid.tile([T, ND, NTOK], bf16, tag="x1T")

        with tc.tile_pool(name="pqkv", bufs=1) as pqkv:
            qT = pqkv.tile([T, ND, NTOK], bf16, tag="qT")
            kT = pqkv.tile([T, ND, NTOKH], bf16, tag="kT")
            v_sb = pqkv.tile([T, NBH, D], bf16, tag="v")
            wo_sb = pqkv.tile([T, ND, D], bf16, tag="wo")
            yT_all = pqkv.tile([T, NB, ND, T], bf16, tag="yT")

            # ================= phase A: x^T, then Q/K/V ====================
            with tc.tile_pool(name="pa", bufs=1) as pa, \
                 tc.tile_pool(name="psA", bufs=2, space="PSUM") as psA:
                wqk_sb = pa.tile([T, ND, DQK], bf16, tag="wqk")
                nc.sync.dma_start(wqk_sb[:],
                                  wqk_d.rearrange("(j p) n -> p j n", p=T))
                wv_sb = pa.tile([T, ND, D], bf16, tag="wv")
                nc.sync.dma_start(wv_sb[:],
                                  wv_d.rearrange("(j p) n -> p j n", p=T))
                # later-phase weights queue behind the phase-A ones
                nc.sync.dma_start(wo_sb[:],
                                  wo_d.rearrange("(j p) n -> p j n", p=T))
                nc.sync.dma_start(w1_sb[:],
                                  w1_d.rearrange("(j p) n -> p j n", p=T))
                xT_all = pa.tile([T, ND, NTOKH], bf16, tag="xT")

                for i in range(NBH):
                    ptr = psA.tile([T, ND, T], bf16, tag="xtr")
                    for j in range(ND):
                        nc.tensor.transpose(ptr[:, j, :],
                                            xh_sb[:, i, j * T:(j + 1) * T],
                                            ident[:])
                    nc.scalar.copy(xT_all[:, :, i * T:(i + 1) * T], ptr[:])

                # Q: own tokens only (2 groups of 512)
                for g in range(2):
                    tsl = slice(T + g * 512, T + (g + 1) * 512)
                    osl = slice(g * 512, (g + 1) * 512)
                    for cc in range(ND):
                        pq = psA.tile([T, 512], f32, tag="aq")
                        for j in range(ND):
                            nc.tensor.matmul(pq[:],
                                             wqk_sb[:, j, cc * T:(cc + 1) * T],
                                             xT_all[:, j, tsl],
                                             start=(j == 0), stop=(j == ND - 1))
                        nc.vector.tensor_scalar_add(qT[:, cc, osl], pq[:],
                                                    qkb_sb[:, cc:cc + 1])
                # K: halo'd tokens (3 groups of 384)
                for g in range(3):
                    tsl = slice(g * 384, (g + 1) * 384)
                    for cc in range(ND):
                        pk = psA.tile([T, 384], f32, tag="ak")
                        for j in range(ND):
                            nc.tensor.matmul(pk[:],
                                             wqk_sb[:, j, D + cc * T:D + (cc + 1) * T],
                                             xT_all[:, j, tsl],
                                             start=(j == 0), stop=(j == ND - 1))
                        nc.scalar.activation(kT[:, cc, tsl], pk[:], AF.Identity,
                                             bias=qkb_sb[:, ND + cc:ND + cc + 1])
                # V: [tok, ch] layout per block (bias folded into obrow)
                for i in range(NBH):
                    pv5 = psA.tile([T, 512], f32, tag="aq")
                    pv2 = psA.tile([T, 256], f32, tag="av2")
                    for j in range(ND):
                        nc.tensor.matmul(pv5[:], xT_all[:, j, i * T:(i + 1) * T],
                                         wv_sb[:, j, 0:512],
                                         start=(j == 0), stop=(j == ND - 1))
                    for j in range(ND):
                        nc.tensor.matmul(pv2[:], xT_all[:, j, i * T:(i + 1) * T],
                                         wv_sb[:, j, 512:768],
                                         start=(j == 0), stop=(j == ND - 1))
                    nc.vector.tensor_copy(v_sb[:, i, 0:512], pv5[:])
                    nc.scalar.copy(v_sb[:, i, 512:768], pv2[:])

            # ====== attention + B1 (out-proj + LN1 + x1^T), interleaved =====
            # PSUM budget (8 banks): st 2x2 + yp 1 + tr 1 + pz 2 = 8
            # QK matmuls grouped by operand partition offset: a po=0 -> 64
            # transition between consecutive matmuls into the same PSUM bank
            # faults on HW, so even heads (po=0) fill bank 0, odd heads bank 1.
            ORDER = (0, 2, 1, 3)
            RPOS = {0: 0, 2: 1, 1: 2, 3: 3}

            def emit_attn(t, attn, psS):
                m01 = m01f_sb if t == 0 else m01r_sb
                y_blk = attn.tile([T, D], bf16, tag="yblk")
                for c3 in range(3):                      # 4 heads per chunk
                    ps_st = psS.tile([T, 4, 2, T], f32, tag="st", bufs=2)
                    for ri, h4 in enumerate(ORDER):
                        h = c3 * 4 + h4
                        cc, po = h // 2, (h % 2) * HD
                        for kb in range(2):
                            nc.tensor.matmul(
                                ps_st[:, ri, kb, :],
                                kT[po:po + HD, cc, (t + kb) * T:(t + kb + 1) * T],
                                qT[po:po + HD, cc, t * T:(t + 1) * T],
                                start=True, stop=True)
                    P = attn.tile([T, 4, 2, T], bf16, tag="P")
                    nc.scalar.activation(P[:], ps_st[:], AF.Exp, scale=0.125)
                    nc.vector.tensor_tensor(P[:], P[:], m01[:], op=ALU.mult)
                    yp = psS.tile([T, 4, 80], f32, tag="yp", bufs=1)
                    nmm = 0
                    for h4 in range(4):
                        h = c3 * 4 + h4
                        ri = RPOS[h4]
                        for kb in range(2):
                            nc.tensor.matmul(
                                yp[:, h4, 0:HD], P[:, ri, kb, :],
                                v_sb[:, t + kb, h * HD:(h + 1) * HD],
                                start=(nmm == 0), stop=False,
                                skip_group_check=True)
                            nmm += 1
                            nc.tensor.matmul(
                                yp[:, h4, HD:HD + 1], P[:, ri, kb, :],
                                ones_col[:],
                                start=False, stop=(nmm == 7),
                                skip_group_check=True)
                            nmm += 1
                    rec = attn.tile([T, 4], f32, tag="rec")
                    nc.vector.reciprocal(rec[:], yp[:, :, HD])
                    for h4 in range(4):
                        h = c3 * 4 + h4
                        nc.vector.tensor_scalar_mul(
                            y_blk[:, h * HD:(h + 1) * HD], yp[:, h4, 0:HD],
                            rec[:, h4:h4 + 1])
                ptr = psS.tile([T, ND, T], bf16, tag="tr", bufs=1)
                for j in range(ND):
                    nc.tensor.transpose(ptr[:, j, :],
                                        y_blk[:, j * T:(j + 1) * T], ident[:])
                nc.vector.tensor_copy(yT_all[:, t, :, :], ptr[:])

            def emit_b1(t, wb, psB):
                pz = psB.tile([T, D], f32, tag="pz", bufs=1)
                for j in range(ND):
                    nc.tensor.matmul(pz[:, 0:512], yT_all[:, t, j, :],
                                     wo_sb[:, j, 0:512],
                                     start=(j == 0), stop=False)
                nc.tensor.matmul(pz[:, 0:512], e0mat[:], obrep_sb[:, 0:512],
                                 start=False, stop=True)
                for j in range(ND):
                    nc.tensor.matmul(pz[:, 512:768], yT_all[:, t, j, :],
                                     wo_sb[:, j, 512:768],
                                     start=(j == 0), stop=False)
                nc.tensor.matmul(pz[:, 512:768], e0mat[:],
                                 obrep_sb[:, 512:768],
                                 start=False, stop=True)
                x1pre = wb.tile([T, D], f32, tag="x1pre")
                s1a = wb.tile([T, 1], f32, tag="s1a")
                s1b = wb.tile([T, 1], f32, tag="s1b")
                nc.vector.scalar_tensor_tensor(
                    x1pre[:, 0:512], pz[:, 0:512], 1.0, xh_sb[:, t + 1, 0:512],
                    op0=ALU.mult, op1=ALU.add, accum_out=s1a[:])
                nc.vector.scalar_tensor_tensor(
                    x1pre[:, 512:768], pz[:, 512:768], 1.0,
                    xh_sb[:, t + 1, 512:768],
                    op0=ALU.mult, op1=ALU.add, accum_out=s1b[:])
                # LN1 (mean from accumulated sums)
                s1 = wb.tile([T, 1], f32, tag="s1")
                nc.vector.tensor_tensor(s1[:], s1a[:], s1b[:], op=ALU.add)
                nm = wb.tile([T, 1], f32, tag="nm")
                nc.scalar.mul(nm[:], s1[:], -1.0 / D)
                xc = wb.tile([T, D], f32, tag="xc")
                nc.vector.tensor_scalar_add(xc[:], x1pre[:], nm[:])
                sq = wb.tile([T, D], f32, tag="sq")
                vs = wb.tile([T, 1], f32, tag="vs")
                nc.vector.scalar_tensor_tensor(sq[:], xc[:], 1.0, xc[:],
                                               op0=ALU.mult, op1=ALU.mult,
                                               accum_out=vs[:])
                std = wb.tile([T, 1], f32, tag="std")
                nc.scalar.activation(std[:], vs[:], AF.Sqrt, bias=eps_sb[:],
                                     scale=1.0 / D)
                rstd = wb.tile([T, 1], f32, tag="rstd")
                nc.vector.reciprocal(rstd[:], std[:])
                nc.vector.scalar_tensor_tensor(x1_all[:, t, :], xc[:], rstd[:],
                                               g1rep_sb[:], op0=ALU.mult,
                                               op1=ALU.mult)
                ptr = psB.tile([T, ND, T], bf16, tag="tr", bufs=1)
                for j in range(ND):
                    nc.tensor.transpose(ptr[:, j, :],
                                        x1_all[:, t, j * T:(j + 1) * T],
                                        ident[:])
                nc.scalar.copy(x1T_all[:, :, t * T:(t + 1) * T], ptr[:])

            with tc.tile_pool(name="attn", bufs=3) as attn, \
                 tc.tile_pool(name="wb1w", bufs=2) as wb, \
                 tc.tile_pool(name="psS", bufs=1, space="PSUM") as psS:
                emit_attn(0, attn, psS)
                for t in range(1, NB):
                    emit_attn(t, attn, psS)
                    emit_b1(t - 1, wb, psS)
                emit_b1(NB - 1, wb, psS)

        # ============= phase B2/B3: FFN + LN2, stage-major ==============
        with tc.tile_pool(name="wB2", bufs=1) as wB2, \
             tc.tile_pool(name="hbuf", bufs=1) as hbuf, \
             tc.tile_pool(name="workB", bufs=2) as workB, \
             tc.tile_pool(name="psF", bufs=2, space="PSUM") as psF:
            w2_sb = wB2.tile([T, NF, D], bf16, tag="w2")
            w2r = w2_d.rearrange("(j p) n -> p j n", p=T)
            nc.sync.dma_start(w2_sb[:, 0:12, :], w2r[:, 0:12, :])
            nc.sync.dma_start(w2_sb[:, 12:24, :], w2r[:, 12:24, :])

            def emit_b2(g, h_g):
                for fi in range(NF):
                    ph = psF.tile([T, 512], f32, tag="ph")
                    for j in range(ND):
                        nc.tensor.matmul(
                            ph[:], w1_sb[:, j, fi * T:(fi + 1) * T],
                            x1T_all[:, j, g * 512:(g + 1) * 512],
                            start=(j == 0), stop=(j == ND - 1))
                    act_fn = AF.Gelu if act == "gelu" else AF.Identity
                    nc.scalar.activation(h_g[:, fi, :], ph[:], act_fn,
                                         bias=gelub_sb[:, fi:fi + 1])

            def emit_b3(t, h_g):
                px5 = psF.tile([T, 512], f32, tag="px5")
                px2 = psF.tile([T, 256], f32, tag="px2")
                tin = (t % 4) * T
                for fi in range(NF):
                    nc.tensor.matmul(px5[:], h_g[:, fi, tin:tin + T],
                                     w2_sb[:, fi, 0:512],
                                     start=(fi == 0), stop=False)
                nc.tensor.matmul(px5[:], e0mat[:], b2rep_sb[:, 0:512],
                                 start=False, stop=True)
                for fi in range(NF):
                    nc.tensor.matmul(px2[:], h_g[:, fi, tin:tin + T],
                                     w2_sb[:, fi, 512:768],
                                     start=(fi == 0), stop=False)
                nc.tensor.matmul(px2[:], e0mat[:], b2rep_sb[:, 512:768],
                                 start=False, stop=True)
                x2pre = workB.tile([T, D], f32, tag="x2pre")
                s1a = workB.tile([T, 1], f32, tag="s1a")
                s1b = workB.tile([T, 1], f32, tag="s1b")
                nc.vector.scalar_tensor_tensor(
                    x2pre[:, 0:512], px5[:], 1.0, x1_all[:, t, 0:512],
                    op0=ALU.mult, op1=ALU.add, accum_out=s1a[:])
                nc.vector.scalar_tensor_tensor(
                    x2pre[:, 512:768], px2[:], 1.0, x1_all[:, t, 512:768],
                    op0=ALU.mult, op1=ALU.add, accum_out=s1b[:])
                s1 = workB.tile([T, 1], f32, tag="s1")
                nc.vector.tensor_tensor(s1[:], s1a[:], s1b[:], op=ALU.add)
                nm = workB.tile([T, 1], f32, tag="nm")
                nc.scalar.mul(nm[:], s1[:], -1.0 / D)
                xc = workB.tile([T, D], f32, tag="xc")
                nc.vector.tensor_scalar_add(xc[:], x2pre[:], nm[:])
                sq = workB.tile([T, D], f32, tag="sq")
                vs = workB.tile([T, 1], f32, tag="vs")
                nc.vector.scalar_tensor_tensor(sq[:], xc[:], 1.0, xc[:],
                                               op0=ALU.mult, op1=ALU.mult,
                                               accum_out=vs[:])
                std = workB.tile([T, 1], f32, tag="std")
                nc.scalar.activation(std[:], vs[:], AF.Sqrt, bias=eps_sb[:],
                                     scale=1.0 / D)
                rstd = workB.tile([T, 1], f32, tag="rstd")
                nc.vector.reciprocal(rstd[:], std[:])
                xg = workB.tile([T, D], f32, tag="xg")
                nc.vector.scalar_tensor_tensor(xg[:], xc[:], rstd[:],
                                               g2rep_sb[:], op0=ALU.mult,
                                               op1=ALU.mult)
                ob = workB.tile([T, D], f32, tag="outb")
                nc.vector.tensor_tensor(ob[:], xg[:], n2brep_sb[:],
                                        op=ALU.add)
                nc.sync.dma_start(out_d[t * T:(t + 1) * T, :], ob[:])

            h_g0 = hbuf.tile([T, NF, 512], bf16, tag="h")
            emit_b2(0, h_g0)
            for t in range(4):
                emit_b3(t, h_g0)
            h_g1 = hbuf.tile([T, NF, 512], bf16, tag="h")
            emit_b2(1, h_g1)
            for t in range(4, NB):
                emit_b3(t, h_g1)

    nc.compile()
    return nc


def _get_nc(act="gelu"):
    if act not in _CACHE:
        _CACHE[act] = _build_nc(act)
    return _CACHE[act]


def make_in_maps(x, in_proj_w, in_proj_b, out_w, out_b, ff_w1, ff_b1,
                 ff_w2, ff_b2, n1_g, n1_b, n2_g, n2_b):
    bf = ml_dtypes.bfloat16
    f32 = np.float32
    x = np.asarray(x, f32).reshape(B, S, D)
    in_proj_w = np.asarray(in_proj_w, f32)
    in_proj_b = np.asarray(in_proj_b, f32)
    out_w = np.asarray(out_w, f32)
    ff_w1 = np.asarray(ff_w1, f32)
    ff_w2 = np.asarray(ff_w2, f32)
    n1_b = np.asarray(n1_b, f32)

    v_b = in_proj_b[DQK:]
    ob_eff = np.asarray(out_b, f32) + v_b @ out_w.T          # V-bias folded
    gelub_eff = np.asarray(ff_b1, f32) + n1_b @ ff_w1.T      # n1_b folded
    b2row_eff = np.asarray(ff_b2, f32) + n1_b                # n1_b residual

    shared = {
        "wqkT": np.ascontiguousarray(in_proj_w[:DQK].T).astype(bf),
        "wvT": np.ascontiguousarray(in_proj_w[DQK:].T).astype(bf),
        "woT": np.ascontiguousarray(out_w.T).astype(bf),
        "w1T": np.ascontiguousarray(ff_w1.T).astype(bf),
        "w2T": np.ascontiguousarray(ff_w2.T).astype(bf),
        "qkb": np.ascontiguousarray(
            in_proj_b[:DQK].reshape(2 * ND, T).T),
        "gelub": np.ascontiguousarray(gelub_eff.reshape(NF, T).T),
        "obrep": np.ascontiguousarray(
            np.broadcast_to(ob_eff[None, :], (T, D))).astype(bf),
        "b2rep": np.ascontiguousarray(
            np.broadcast_to(b2row_eff[None, :], (T, D))).astype(bf),
        "g1rep": np.ascontiguousarray(
            np.broadcast_to(np.asarray(n1_g, f32)[None, :], (T, D))),
        "g2rep": np.ascontiguousarray(
            np.broadcast_to(np.asarray(n2_g, f32)[None, :], (T, D))),
        "n2brep": np.ascontiguousarray(
            np.broadcast_to(np.asarray(n2_b, f32)[None, :], (T, D))),
    }

    # masks in [k, (h4, kb, q)] layout, 0/1 bf16, tiled over 4 heads
    k_i = np.arange(T, dtype=np.int64)[:, None]
    q_i = np.arange(T, dtype=np.int64)[None, :]
    m_kb0 = (k_i > q_i).astype(f32)         # previous key block
    m_kb1 = (k_i <= q_i).astype(f32)        # current key block (causal)
    rest = np.concatenate([m_kb0, m_kb1], axis=1)          # [T, 2T]
    first = np.concatenate([np.zeros((T, T), f32), m_kb1], axis=1)
    m01r = np.ascontiguousarray(np.tile(rest, (1, 4))).astype(bf)
    m01f_bs = np.ascontiguousarray(np.tile(first, (1, 4))).astype(bf)

    in_maps = []
    for c in range(N_CORES):
        b, i0 = divmod(c * NTOK, S)
        halo = (np.zeros((T, D), f32) if i0 == 0 else x[b, i0 - T:i0])
        xh = np.ascontiguousarray(
            np.concatenate([halo, x[b, i0:i0 + NTOK]], axis=0)).astype(bf)
        m = dict(shared)
        m["xh"] = xh
        m["m01f"] = m01f_bs if i0 == 0 else m01r
        m["m01r"] = m01r
        in_maps.append(m)
    return in_maps


def kernel(**inputs):
    from concourse.bass_utils import run_bass_kernel_spmd
    nc = _get_nc()
    in_maps = make_in_maps(**inputs)
    res = run_bass_kernel_spmd(nc, in_maps, core_ids=list(range(N_CORES)))
    outs = [res.results[c]["out"] for c in range(N_CORES)]
    return np.concatenate(outs, axis=0).reshape(B, S, D).astype(np.float32)
